# revision 50
# baseline (speedup 1.0000x reference)
"""Trainium2 Bass kernel for BaseAttention (B=4, T=2048, D=1024, H=16, K=4).

Sharding: 8 cores = 4 batches x 2 head-groups (8 heads each).
Per core: qkv projection (bf16 matmuls, fp32 accumulate), causal attention in
transposed-score layout (softmax denominators via a ones-column in the V
operand, causal mask folded into the PE as an identity x (-1e30 triangle)
accumulate), embedding gather bias (transposed on the PE), and a bf16
out-projection over this head-group's columns. Host sums the two partial
outputs per batch and adds bout.

Schedule: the attention phase is paced by the ACT-engine exp stream, so
 - chunk 0's attention (pure diagonal) is emitted inside the projection
   phase, hiding its exps under the PE-dense qkv matmuls;
 - off-diagonal attention runs in fp8e4 DoubleRow mode: score j-block pairs
   share one 2-bank PSUM tile, a single merged exp writes the fp8 weight
   pair, one DoubleRow matmul contracts 256 keys at 0.5 cyc/row (noise
   averages down over >=512 keys; diagonals stay bf16 for early tokens);
 - exp inputs are biased by -4 so unnormalized weights stay inside fp8e4
   range (max 240); the ones-column denominator absorbs the same factor;
 - bias-transpose + out-projection work is queued as ~0.5us steps and doled
   out between attention units as PE fill-in, never in front of the next
   unit's score matmuls.
"""
import sys

sys.path.insert(0, "/opt/trn_rl_repo")

import numpy as np
import ml_dtypes
import concourse.bass as bass
import concourse.mybir as mybir
import concourse.tile as tile
from concourse import bacc
from concourse.bass_utils import run_bass_kernel_spmd

P = 128
B, T, D, H = 4, 2048, 1024, 16
HD = D // H          # 64
KT = 4               # templates per token
VOCAB = 32000
G = 512              # columns per head-group (8 heads x 64)
NCORES = 8
TB = T // P          # 16 t-blocks of 128
TC = T // 512        # 4 t-chunks of 512
DK = D // P          # 8 contraction blocks
NEG = -1.0e30

F32 = mybir.dt.float32
BF16 = mybir.dt.bfloat16
F8 = mybir.dt.float8e4
I32 = mybir.dt.int32
Copy = mybir.ActivationFunctionType.Copy
Exp = mybir.ActivationFunctionType.Exp
DR = mybir.MatmulPerfMode.DoubleRow

_NC_CACHE = None


def _build():
    nc = bacc.Bacc("TRN2", target_bir_lowering=False, debug=False,
                   num_devices=NCORES)

    xT = nc.dram_tensor("xT", [D, T], BF16, kind="ExternalInput").ap()
    wqk = nc.dram_tensor("wqk", [D, 1024], BF16, kind="ExternalInput").ap()
    wv = nc.dram_tensor("wv", [D, G], BF16, kind="ExternalInput").ap()
    wout = nc.dram_tensor("wout", [G, D], BF16, kind="ExternalInput").ap()
    embs = nc.dram_tensor("embs", [VOCAB, G], F32, kind="ExternalInput").ap()
    seqs = nc.dram_tensor("seqs", [T, KT], I32, kind="ExternalInput").ap()
    bigmask = nc.dram_tensor("bigmask", [P, 896], F32, kind="ExternalInput").ap()
    identin = nc.dram_tensor("identin", [P, P], F32, kind="ExternalInput").ap()
    y = nc.dram_tensor("y", [T, D], BF16, kind="ExternalOutput").ap()

    with tile.TileContext(nc) as tc:
        with (
            tc.tile_pool(name="persist", bufs=1) as pp,
            tc.tile_pool(name="qk", bufs=1) as pqk,
            tc.tile_pool(name="v1", bufs=1) as pv1,
            tc.tile_pool(name="gath", bufs=6) as pg,
            tc.tile_pool(name="ebias", bufs=16) as pe,
            tc.tile_pool(name="work", bufs=1) as pw,
            tc.tile_pool(name="wtp", bufs=8) as pwt,
            tc.tile_pool(name="recp", bufs=2) as prec,
            tc.tile_pool(name="bcsp", bufs=4) as pbcs,
            tc.tile_pool(name="pssc", bufs=2, space="PSUM") as pssc,
            tc.tile_pool(name="psmm", bufs=2, space="PSUM") as psmm,
            tc.tile_pool(name="psot", bufs=2, space="PSUM") as psot,
        ):
            # ---- PE warmup spin during the DMA ramp: ~5us of back-to-back
            # matmuls on a zeroed tile pushes the HAM clock gate to 8/8 before
            # the real qkv matmuls start.
            wz = pp.tile([P, 512], BF16, tag="wz", name="wz")
            nc.vector.memset(wz[:], 0.0)
            wzp = psmm.tile([P, 512], F32, tag="mm", name="wzp")
            for w in range(28):
                nc.tensor.matmul(wzp[:], wz[:, 0:P], wz[:],
                                 start=(w == 0), stop=(w == 27))
            wzs = pp.tile([P, 512], F32, tag="wzs", name="wzs")
            nc.scalar.activation(wzs[:], wzp[:], Copy)

            # ---- persistent small tensors
            mask_sb = pp.tile([P, 896], F32, tag="mask", name="mask")
            ident = pp.tile([P, P], F32, tag="ident", name="ident")
            ones_sb = pp.tile([1, 64], BF16, tag="ones", name="ones")
            nc.vector.memset(ones_sb[:], 1.0)
            # exp bias -4: keeps unnormalized softmax weights inside fp8e4
            # range (max normal 240); the ones-column denominator picks up the
            # same e^-4 factor, so normalization cancels it exactly.
            cneg4 = pp.tile([P, 1], F32, tag="cneg4", name="cneg4")
            nc.vector.memset(cneg4[:], -4.0)
            ident_bf = pp.tile([P, P], BF16, tag="identbf", name="identbf")
            maskneg_bf = pp.tile([P, P], BF16, tag="masknegbf", name="masknegbf")
            wout_sb = [pp.tile([P, D], BF16, tag=f"wout{pb}", name=f"wout{pb}")
                       for pb in range(4)]

            # qkT: 8 pblocks (4 q + 4 k), [128 feat, T] bf16
            qkT = [pqk.tile([P, T], BF16, tag=f"qk{j}", name=f"qk{j}") for j in range(8)]
            # v1: 16 t-blocks [128, 8 heads, 65] bf16 (col 64 = ones)
            v1 = [pv1.tile([P, 8, HD + 1], BF16, tag=f"v{t}", name=f"v{t}") for t in range(TB)]
            # v8: fp8 copy for DoubleRow AV, paired t-blocks [key, head, jsub, 80]
            # (only t-blocks 0..11 are ever off-diagonal; inner dim padded
            # 65->80 because DoubleRow ldweights needs k-subtile step % 16 == 0)
            v8 = [pv1.tile([P, 8, 2, 80], F8, tag=f"v8{t}", name=f"v8{t}")
                  for t in range(6)]
            for t in range(6):
                nc.vector.memset(v8[t][:, :, :, HD:HD + 1], 1.0)
            # attention output, transposed [feat, T] bf16
            aT = [pw.tile([P, T], BF16, tag=f"a{pb}", name=f"a{pb}") for pb in range(4)]

            # ============ attention unit machinery ============
            def emit_attn(ic, hp, w8pool, emitfn):
                i0 = ic * 512
                qt, kt = qkT[hp], qkT[4 + hp]
                ot = [psot.tile([P, 512], F32, tag="ot", name=f"ot{s}")
                      for s in range(2)]
                npair = 2 * ic       # off-diagonal j-block pairs
                nunit = npair + 2    # + two diagonal double-units

                def scores(u):
                    wts = []
                    for s in range(2):
                        hbase = s * HD
                        sc = pssc.tile([P, 1024], F32, tag="sc", name="sc")
                        if u < npair:
                            for j in range(2):
                                jb = 2 * u + j
                                nc.tensor.matmul(
                                    sc[:, j * 512:(j + 1) * 512],
                                    kt[hbase:hbase + HD, jb * P:(jb + 1) * P],
                                    qt[hbase:hbase + HD, i0:i0 + 512],
                                    start=True, stop=True,
                                    tile_position=(hbase, 0))
                            # merged exp over both j-blocks, fp8 out
                            w8 = w8pool.tile([P, 2, 512], F8, tag="w8",
                                             name="w8")
                            nc.scalar.activation(
                                w8[:].rearrange("p a b -> p (a b)"),
                                sc[:], Exp, bias=cneg4[:], scale=0.125)
                            wts.append(w8)
                        else:
                            pairw = []
                            for j in range(2):
                                r = 2 * (u - npair) + j
                                lo = r * P
                                co = j * 512
                                nc.tensor.matmul(
                                    sc[:, co + lo:co + 512],
                                    kt[hbase:hbase + HD,
                                       (4 * ic + r) * P:(4 * ic + r + 1) * P],
                                    qt[hbase:hbase + HD, i0 + lo:i0 + 512],
                                    start=True, stop=True,
                                    tile_position=(hbase, 0))
                                nc.tensor.matmul(
                                    sc[:, co + lo:co + lo + P],
                                    ident_bf[:], maskneg_bf[:],
                                    start=False, stop=True,
                                    skip_group_check=True)
                                wt = pwt.tile([P, 512], BF16, tag="wt",
                                              name="wt")
                                nc.scalar.activation(
                                    wt[:, lo:512], sc[:, co + lo:co + 512],
                                    Exp, bias=cneg4[:], scale=0.125)
                                pairw.append((wt, lo, r))
                            wts.append(pairw)
                    return wts

                def accum(u, wts):
                    for s in range(2):
                        if u < npair:
                            nc.tensor.matmul(
                                ot[s][0:HD + 1, :],
                                v8[u][:, 2 * hp + s, :, 0:HD + 1],
                                wts[s][:],
                                start=(u == 0), stop=False,
                                perf_mode=DR)
                        else:
                            for (wt, lo, r) in wts[s]:
                                jb = 4 * ic + r
                                nc.tensor.matmul(
                                    ot[s][0:HD + 1, lo:512],
                                    v1[jb][:, 2 * hp + s, :],
                                    wt[:, lo:512],
                                    start=(u == 0 and r == 0),
                                    stop=(r == 3))

                prev = scores(0)
                for u in range(1, nunit):
                    cur = scores(u)
                    accum(u - 1, prev)
                    emitfn()
                    if len(filler) > 10:
                        emitfn()
                    prev = cur
                accum(nunit - 1, prev)
                # normalize: aT rows = o / denom (both reciprocals issued
                # first so the second's DVE latency hides under the first's
                # broadcast + multiply)
                recs = []
                for s in range(2):
                    # bf16: an fp32 moving operand costs 4.0 cyc/row on the
                    # PE (853ns vs 213ns for this broadcast matmul); bf16
                    # reciprocal quantization (~0.4%) scales a whole token's
                    # attention row uniformly, well inside the error budget.
                    rec = prec.tile([1, 512], BF16, tag="rec", name="rec")
                    with nc.allow_low_precision(reason="bf16 softmax recip"):
                        nc.vector.reciprocal(rec[:], ot[s][HD:HD + 1, :])
                    recs.append(rec)
                for s in range(2):
                    bc = psmm.tile([P, 512], F32, tag="mm", name="bc")
                    nc.tensor.matmul(bc[0:HD, :], ones_sb[:], recs[s][:],
                                     start=True, stop=True)
                    bcs = pbcs.tile([HD, 512], F32, tag="bcs", name="bcs")
                    nc.vector.tensor_copy(bcs[:], bc[0:HD, :])
                    nc.vector.tensor_tensor(
                        aT[hp][s * HD:(s + 1) * HD, i0:i0 + 512],
                        ot[s][0:HD, :], bcs[:],
                        mybir.AluOpType.mult)

            noop = lambda: None
            ebs = []

            # ================= phase 1: qkv projections =================
            # chunk-0 attention (pure diagonal) is emitted mid-phase: its exp
            # stream runs on the otherwise idle ACT engine while PE grinds
            # through the projection matmuls.
            import contextlib
            _stack = contextlib.ExitStack()
            pxt = _stack.enter_context(tc.tile_pool(name="xt", bufs=1))
            if True:
                xt = [pxt.tile([P, T], BF16, tag=f"x{k}", name=f"x{k}") for k in range(DK)]
                wv_sb = [pxt.tile([P, G], BF16, tag=f"wv{k}", name=f"wv{k}") for k in range(DK)]
                wqk_sb = [pxt.tile([P, 1024], BF16, tag=f"wqk{k}", name=f"wqk{k}")
                          for k in range(DK)]
                # head loads: only x cols 0:512 gate the first v/qk blocks
                # (attention chunk 0 and all t<4 v-blocks live there); the
                # x remainder streams in under the projection compute.
                for k in range(DK):
                    nc.sync.dma_start(wv_sb[k][:], wv[k * P:(k + 1) * P, :])
                for k in range(DK):
                    nc.sync.dma_start(xt[k][:, 0:512],
                                      xT[k * P:(k + 1) * P, 0:512])
                for k in range(DK):
                    nc.sync.dma_start(wqk_sb[k][:], wqk[k * P:(k + 1) * P, :])
                for k in range(DK):
                    nc.sync.dma_start(xt[k][:, 512:T],
                                      xT[k * P:(k + 1) * P, 512:T])
                nc.sync.dma_start(mask_sb[:], bigmask[:])
                nc.sync.dma_start(ident[:], identin[:])
                nc.vector.tensor_copy(ident_bf[:], ident[:])
                nc.vector.tensor_copy(maskneg_bf[:], mask_sb[:, 384:512])
                for pb in range(4):
                    nc.sync.dma_start(wout_sb[pb][:], wout[pb * P:(pb + 1) * P, :])

                def v_block(t):
                    ps = psmm.tile([P, G], F32, tag="mm", name="vps")
                    for k in range(DK):
                        nc.tensor.matmul(
                            ps[:], xt[k][:, t * P:(t + 1) * P], wv_sb[k][:],
                            start=(k == 0), stop=(k == DK - 1))
                    nc.scalar.activation(
                        v1[t][:, :, 0:HD],
                        ps[:].rearrange("p (h c) -> p h c", h=8), Copy)
                    nc.vector.memset(v1[t][:, :, HD:HD + 1], 1.0)
                    if t < 12:
                        # fp8 copy for DoubleRow AV. t<4 runs on Pool
                        # (emitted before the gather burst); later blocks are
                        # emitted mid-attention where Pool is still chewing
                        # gather descriptors, so they go to DVE instead.
                        eng = nc.gpsimd if t < 4 else nc.vector
                        eng.tensor_copy(
                            v8[t // 2][:, :, t % 2, 0:HD], v1[t][:, :, 0:HD])

                def qk_block_c(jp, c):
                    ps = psmm.tile([P, 512], F32, tag="mm", name="qkps")
                    for k in range(DK):
                        nc.tensor.matmul(
                            ps[:], wqk_sb[k][:, jp * P:(jp + 1) * P],
                            xt[k][:, c * 512:(c + 1) * 512],
                            start=(k == 0), stop=(k == DK - 1))
                    nc.vector.tensor_copy(
                        qkT[jp][:, c * 512:(c + 1) * 512], ps[:])

                def gather_block(t):
                    idxt = pe.tile([P, KT], I32, tag="idx", name="idxt")
                    nc.sync.dma_start(idxt[:], seqs[t * P:(t + 1) * P, :])
                    gt = [pg.tile([P, G], F32, tag="g", name=f"g{kk}")
                          for kk in range(KT)]
                    for kk in range(KT):
                        nc.gpsimd.indirect_dma_start(
                            out=gt[kk][:], out_offset=None, in_=embs[:],
                            in_offset=bass.IndirectOffsetOnAxis(
                                ap=idxt[:, kk:kk + 1], axis=0))
                    ga = pg.tile([P, G], F32, tag="g", name="gs01")
                    nc.vector.tensor_add(ga[:], gt[0][:], gt[1][:])
                    gb = pg.tile([P, G], F32, tag="g", name="gs23")
                    nc.vector.tensor_add(gb[:], gt[2][:], gt[3][:])
                    eb = pe.tile([P, G], BF16, tag="eb", name="eb")
                    nc.vector.tensor_add(eb[:], ga[:], gb[:])
                    ebs.append(eb)

                for t in range(4):
                    v_block(t)
                for jp in (0, 4, 1, 5, 2, 6, 3, 7):
                    qk_block_c(jp, 0)
                for t in range(TB):
                    gather_block(t)

            # ============ phase 2: attention + bias + out-proj ============
            with (
                tc.tile_pool(name="w8p", bufs=5) as pw8,
                tc.tile_pool(name="yout", bufs=3) as py,
            ):
                # tail work (bias transpose-add + out-projection + store) is
                # queued as ~0.5us steps and interleaved between attention
                # units: keeps PE fed during the ACT-paced exp stream without
                # parking a large PE block in front of the next unit's score
                # matmuls (which would starve ACT).
                filler = []

                def queue_tail(t):
                    def step_tp():
                        for pb in range(4):
                            tp = psmm.tile([P, P], BF16, tag="mm", name="tp")
                            nc.tensor.transpose(
                                tp[:], ebs[t][:, pb * P:(pb + 1) * P],
                                ident_bf[:])
                            nc.vector.tensor_add(
                                aT[pb][:, t * P:(t + 1) * P],
                                aT[pb][:, t * P:(t + 1) * P], tp[:])
                    box = {}

                    def step_proj(ch, half):
                        def go():
                            if ch == 0 and half == 0:
                                box["ysb"] = py.tile([P, D], BF16, tag="y",
                                                     name="y")
                            if half == 0:
                                box[ch] = psmm.tile([P, 512], F32, tag="mm",
                                                    name="yps")
                            yp = box[ch]
                            for pb in (0, 1) if half == 0 else (2, 3):
                                nc.tensor.matmul(
                                    yp[:],
                                    aT[pb][:, t * P:(t + 1) * P],
                                    wout_sb[pb][:, ch * 512:(ch + 1) * 512],
                                    start=(pb == 0), stop=(pb == 3))
                            if half == 1:
                                ysb = box["ysb"]
                                nc.vector.tensor_copy(
                                    ysb[:, ch * 512:(ch + 1) * 512], yp[:])
                                if ch == 1:
                                    nc.sync.dma_start(y[t * P:(t + 1) * P, :],
                                                      ysb[:])
                        return go

                    filler.append(step_tp)
                    for ch in range(2):
                        filler.append(step_proj(ch, 0))
                        filler.append(step_proj(ch, 1))

                def emit_filler():
                    if filler:
                        filler.pop(0)()

                # projection work for later chunks, in dependency order:
                # attention chunk ic needs q-chunk ic, k-chunks <= ic and
                # v t-blocks <= 4*ic+3. Emitted as ~2us filler steps under
                # the ACT-paced exp stream; forced drains before each
                # emit_attn guarantee the deps are in the PE queue.
                need = {}
                for ic in range(1, TC):
                    for hp in range(4):
                        if hp == 0:
                            filler.append(lambda ic=ic: qk_block_c(0, ic))
                            filler.append(lambda ic=ic: qk_block_c(4, ic))
                            for t in range(4 * ic, 4 * ic + 4):
                                filler.append(lambda t=t: v_block(t))
                        else:
                            filler.append(
                                lambda hp=hp, ic=ic: qk_block_c(hp, ic))
                            filler.append(
                                lambda hp=hp, ic=ic: qk_block_c(4 + hp, ic))
                        need[(ic, hp)] = len(filler)
                emitted = [0]

                def emit_filler():
                    if filler:
                        filler.pop(0)()
                        emitted[0] += 1

                def drain_to(n):
                    while emitted[0] < n and filler:
                        emit_filler()

                for hp in range(4):
                    emit_attn(0, hp, pw8, emit_filler)
                for ic in range(1, TC):
                    for t in range(4 * (ic - 1), 4 * ic):
                        queue_tail(t)
                    for hp in range(4):
                        drain_to(need[(ic, hp)])
                        emit_attn(ic, hp, pw8, emit_filler)

                while filler:
                    emit_filler()
                qs = []
                for t in range(4 * (TC - 1), 4 * TC):
                    queue_tail(t)
                    qs.append(list(filler))
                    filler.clear()
                for step in range(max(len(q) for q in qs)):
                    for q in qs:
                        if step < len(q):
                            q[step]()
            _stack.close()

    nc.finalize()
    return nc


def _get_nc():
    global _NC_CACHE
    if _NC_CACHE is None:
        _NC_CACHE = _build()
    return _NC_CACHE


def _make_bigmask():
    # bigmask[jj, u] = NEG if u < jj + 384 else 0; mask for diagonal square of
    # block r is bigmask[:, 384:512] after shifting scores slice by r*128.
    jj = np.arange(P)[:, None]
    u = np.arange(896)[None, :]
    return np.where(u < jj + 384, np.float32(NEG), np.float32(0.0))


def _stage_inputs(x, sequences, Wqkv, Wout, emb, bias_scale, bout=None):
    x = np.asarray(x, dtype=np.float32)
    sequences = np.asarray(sequences)
    Wqkv = np.asarray(Wqkv, dtype=np.float32)
    Wout = np.asarray(Wout, dtype=np.float32)
    emb = np.asarray(emb, dtype=np.float32)
    bs = np.float32(np.asarray(bias_scale))

    bigmask = _make_bigmask()
    identm = np.eye(P, dtype=np.float32)

    in_maps = []
    for core in range(NCORES):
        b, g = divmod(core, 2)
        q_rows = slice(g * G, (g + 1) * G)
        k_rows = slice(D + g * G, D + (g + 1) * G)
        v_rows = slice(2 * D + g * G, 2 * D + (g + 1) * G)
        cols = slice(g * G, (g + 1) * G)
        in_maps.append(dict(
            xT=np.ascontiguousarray(x[b].T).astype(ml_dtypes.bfloat16),
            wqk=np.ascontiguousarray(
                np.concatenate([Wqkv[q_rows].T, Wqkv[k_rows].T],
                               axis=1)).astype(ml_dtypes.bfloat16),
            wv=np.ascontiguousarray(Wqkv[v_rows].T).astype(ml_dtypes.bfloat16),
            wout=np.ascontiguousarray(Wout[:, cols].T).astype(ml_dtypes.bfloat16),
            embs=np.ascontiguousarray(emb[:, cols] * bs),
            seqs=np.ascontiguousarray(sequences[b].astype(np.int32)),
            bigmask=bigmask,
            identin=identm,
        ))
    return in_maps


def _run(inputs, trace=False):
    nc = _get_nc()
    in_maps = _stage_inputs(**inputs)
    res = run_bass_kernel_spmd(nc, in_maps, core_ids=list(range(NCORES)),
                               trace=trace)
    bout = np.asarray(inputs["bout"], dtype=np.float32)
    out = np.empty((B, T, D), dtype=np.float32)
    for b in range(B):
        out[b] = (res.results[2 * b]["y"].astype(np.float32)
                  + res.results[2 * b + 1]["y"].astype(np.float32) + bout)
    return out, res


def kernel(**inputs):
    out, _ = _run(inputs, trace=False)
    return out


# revision 51
# speedup vs baseline: 1.0064x; 1.0064x over previous
"""Trainium2 Bass kernel for BaseAttention (B=4, T=2048, D=1024, H=16, K=4).

Sharding: 8 cores = 4 batches x 2 head-groups (8 heads each).
Per core: qkv projection (bf16 matmuls, fp32 accumulate), causal attention in
transposed-score layout (softmax denominators via a ones-column in the V
operand, causal mask folded into the PE as an identity x (-1e30 triangle)
accumulate), embedding gather bias (transposed on the PE), and a bf16
out-projection over this head-group's columns. Host sums the two partial
outputs per batch and adds bout.

Schedule: the attention phase is paced by the ACT-engine exp stream, so
 - chunk 0's attention (pure diagonal) is emitted inside the projection
   phase, hiding its exps under the PE-dense qkv matmuls;
 - off-diagonal attention runs in fp8e4 DoubleRow mode: score j-block pairs
   share one 2-bank PSUM tile, a single merged exp writes the fp8 weight
   pair, one DoubleRow matmul contracts 256 keys at 0.5 cyc/row (noise
   averages down over >=512 keys; diagonals stay bf16 for early tokens);
 - exp inputs are biased by -4 so unnormalized weights stay inside fp8e4
   range (max 240); the ones-column denominator absorbs the same factor;
 - bias-transpose + out-projection work is queued as ~0.5us steps and doled
   out between attention units as PE fill-in, never in front of the next
   unit's score matmuls.
"""
import sys

sys.path.insert(0, "/opt/trn_rl_repo")

import numpy as np
import ml_dtypes
import concourse.bass as bass
import concourse.mybir as mybir
import concourse.tile as tile
from concourse import bacc
from concourse.bass_utils import run_bass_kernel_spmd

P = 128
B, T, D, H = 4, 2048, 1024, 16
HD = D // H          # 64
KT = 4               # templates per token
VOCAB = 32000
G = 512              # columns per head-group (8 heads x 64)
NCORES = 8
TB = T // P          # 16 t-blocks of 128
TC = T // 512        # 4 t-chunks of 512
DK = D // P          # 8 contraction blocks
NEG = -1.0e30

F32 = mybir.dt.float32
BF16 = mybir.dt.bfloat16
F8 = mybir.dt.float8e4
I32 = mybir.dt.int32
Copy = mybir.ActivationFunctionType.Copy
Exp = mybir.ActivationFunctionType.Exp
DR = mybir.MatmulPerfMode.DoubleRow

_NC_CACHE = None


def _build():
    nc = bacc.Bacc("TRN2", target_bir_lowering=False, debug=False,
                   num_devices=NCORES)

    xT = nc.dram_tensor("xT", [D, T], BF16, kind="ExternalInput").ap()
    wqk = nc.dram_tensor("wqk", [D, 1024], BF16, kind="ExternalInput").ap()
    wv = nc.dram_tensor("wv", [D, G], BF16, kind="ExternalInput").ap()
    wout = nc.dram_tensor("wout", [G, D], BF16, kind="ExternalInput").ap()
    embs = nc.dram_tensor("embs", [VOCAB, G], F32, kind="ExternalInput").ap()
    seqs = nc.dram_tensor("seqs", [T, KT], I32, kind="ExternalInput").ap()
    bigmask = nc.dram_tensor("bigmask", [P, 896], F32, kind="ExternalInput").ap()
    identin = nc.dram_tensor("identin", [P, P], F32, kind="ExternalInput").ap()
    y = nc.dram_tensor("y", [T, D], BF16, kind="ExternalOutput").ap()

    with tile.TileContext(nc) as tc:
        with (
            tc.tile_pool(name="persist", bufs=1) as pp,
            tc.tile_pool(name="qk", bufs=1) as pqk,
            tc.tile_pool(name="v1", bufs=1) as pv1,
            tc.tile_pool(name="gath", bufs=6) as pg,
            tc.tile_pool(name="ebias", bufs=16) as pe,
            tc.tile_pool(name="work", bufs=1) as pw,
            tc.tile_pool(name="wtp", bufs=8) as pwt,
            tc.tile_pool(name="recp", bufs=2) as prec,
            tc.tile_pool(name="bcsp", bufs=4) as pbcs,
            tc.tile_pool(name="pssc", bufs=2, space="PSUM") as pssc,
            tc.tile_pool(name="psmm", bufs=2, space="PSUM") as psmm,
            tc.tile_pool(name="psot", bufs=2, space="PSUM") as psot,
        ):
            # ---- PE warmup spin during the DMA ramp: ~5us of back-to-back
            # matmuls on a zeroed tile pushes the HAM clock gate to 8/8 before
            # the real qkv matmuls start.
            wz = pp.tile([P, 512], BF16, tag="wz", name="wz")
            nc.vector.memset(wz[:], 0.0)
            wzp = psmm.tile([P, 512], F32, tag="mm", name="wzp")
            for w in range(24):
                nc.tensor.matmul(wzp[:], wz[:, 0:P], wz[:],
                                 start=(w == 0), stop=(w == 23))
            wzs = pp.tile([P, 512], F32, tag="wzs", name="wzs")
            nc.scalar.activation(wzs[:], wzp[:], Copy)

            # ---- persistent small tensors
            mask_sb = pp.tile([P, 896], F32, tag="mask", name="mask")
            ident = pp.tile([P, P], F32, tag="ident", name="ident")
            ones_sb = pp.tile([1, 64], BF16, tag="ones", name="ones")
            nc.vector.memset(ones_sb[:], 1.0)
            # exp bias -4: keeps unnormalized softmax weights inside fp8e4
            # range (max normal 240); the ones-column denominator picks up the
            # same e^-4 factor, so normalization cancels it exactly.
            cneg4 = pp.tile([P, 1], F32, tag="cneg4", name="cneg4")
            nc.vector.memset(cneg4[:], -4.0)
            ident_bf = pp.tile([P, P], BF16, tag="identbf", name="identbf")
            maskneg_bf = pp.tile([P, P], BF16, tag="masknegbf", name="masknegbf")
            wout_sb = [pp.tile([P, D], BF16, tag=f"wout{pb}", name=f"wout{pb}")
                       for pb in range(4)]

            # qkT: 8 pblocks (4 q + 4 k), [128 feat, T] bf16
            qkT = [pqk.tile([P, T], BF16, tag=f"qk{j}", name=f"qk{j}") for j in range(8)]
            # v1: 16 t-blocks [128, 8 heads, 65] bf16 (col 64 = ones)
            v1 = [pv1.tile([P, 8, HD + 1], BF16, tag=f"v{t}", name=f"v{t}") for t in range(TB)]
            # v8: fp8 copy for DoubleRow AV, paired t-blocks [key, head, jsub, 80]
            # (only t-blocks 0..11 are ever off-diagonal; inner dim padded
            # 65->80 because DoubleRow ldweights needs k-subtile step % 16 == 0)
            v8 = [pv1.tile([P, 8, 2, 80], F8, tag=f"v8{t}", name=f"v8{t}")
                  for t in range(6)]
            for t in range(6):
                nc.vector.memset(v8[t][:, :, :, HD:HD + 1], 1.0)
            # attention output, transposed [feat, T] bf16
            aT = [pw.tile([P, T], BF16, tag=f"a{pb}", name=f"a{pb}") for pb in range(4)]

            # ============ attention unit machinery ============
            def emit_attn(ic, hp, w8pool, emitfn):
                i0 = ic * 512
                qt, kt = qkT[hp], qkT[4 + hp]
                ot = [psot.tile([P, 512], F32, tag="ot", name=f"ot{s}")
                      for s in range(2)]
                npair = 2 * ic       # off-diagonal j-block pairs
                nunit = npair + 2    # + two diagonal double-units

                def scores(u):
                    wts = []
                    for s in range(2):
                        hbase = s * HD
                        sc = pssc.tile([P, 1024], F32, tag="sc", name="sc")
                        if u < npair:
                            for j in range(2):
                                jb = 2 * u + j
                                nc.tensor.matmul(
                                    sc[:, j * 512:(j + 1) * 512],
                                    kt[hbase:hbase + HD, jb * P:(jb + 1) * P],
                                    qt[hbase:hbase + HD, i0:i0 + 512],
                                    start=True, stop=True,
                                    tile_position=(hbase, 0))
                            # merged exp over both j-blocks, fp8 out
                            w8 = w8pool.tile([P, 2, 512], F8, tag="w8",
                                             name="w8")
                            nc.scalar.activation(
                                w8[:].rearrange("p a b -> p (a b)"),
                                sc[:], Exp, bias=cneg4[:], scale=0.125)
                            wts.append(w8)
                        else:
                            pairw = []
                            for j in range(2):
                                r = 2 * (u - npair) + j
                                lo = r * P
                                co = j * 512
                                nc.tensor.matmul(
                                    sc[:, co + lo:co + 512],
                                    kt[hbase:hbase + HD,
                                       (4 * ic + r) * P:(4 * ic + r + 1) * P],
                                    qt[hbase:hbase + HD, i0 + lo:i0 + 512],
                                    start=True, stop=True,
                                    tile_position=(hbase, 0))
                                nc.tensor.matmul(
                                    sc[:, co + lo:co + lo + P],
                                    ident_bf[:], maskneg_bf[:],
                                    start=False, stop=True,
                                    skip_group_check=True)
                                wt = pwt.tile([P, 512], BF16, tag="wt",
                                              name="wt")
                                nc.scalar.activation(
                                    wt[:, lo:512], sc[:, co + lo:co + 512],
                                    Exp, bias=cneg4[:], scale=0.125)
                                pairw.append((wt, lo, r))
                            wts.append(pairw)
                    return wts

                def accum(u, wts):
                    for s in range(2):
                        if u < npair:
                            nc.tensor.matmul(
                                ot[s][0:HD + 1, :],
                                v8[u][:, 2 * hp + s, :, 0:HD + 1],
                                wts[s][:],
                                start=(u == 0), stop=False,
                                perf_mode=DR)
                        else:
                            for (wt, lo, r) in wts[s]:
                                jb = 4 * ic + r
                                nc.tensor.matmul(
                                    ot[s][0:HD + 1, lo:512],
                                    v1[jb][:, 2 * hp + s, :],
                                    wt[:, lo:512],
                                    start=(u == 0 and r == 0),
                                    stop=(r == 3))

                prev = scores(0)
                for u in range(1, nunit):
                    cur = scores(u)
                    accum(u - 1, prev)
                    emitfn()
                    if len(filler) > 10:
                        emitfn()
                    prev = cur
                accum(nunit - 1, prev)
                # normalize: aT rows = o / denom (both reciprocals issued
                # first so the second's DVE latency hides under the first's
                # broadcast + multiply)
                recs = []
                for s in range(2):
                    # bf16: an fp32 moving operand costs 4.0 cyc/row on the
                    # PE (853ns vs 213ns for this broadcast matmul); bf16
                    # reciprocal quantization (~0.4%) scales a whole token's
                    # attention row uniformly, well inside the error budget.
                    rec = prec.tile([1, 512], BF16, tag="rec", name="rec")
                    with nc.allow_low_precision(reason="bf16 softmax recip"):
                        nc.vector.reciprocal(rec[:], ot[s][HD:HD + 1, :])
                    recs.append(rec)
                for s in range(2):
                    bc = psmm.tile([P, 512], F32, tag="mm", name="bc")
                    nc.tensor.matmul(bc[0:HD, :], ones_sb[:], recs[s][:],
                                     start=True, stop=True)
                    bcs = pbcs.tile([HD, 512], F32, tag="bcs", name="bcs")
                    nc.vector.tensor_copy(bcs[:], bc[0:HD, :])
                    nc.vector.tensor_tensor(
                        aT[hp][s * HD:(s + 1) * HD, i0:i0 + 512],
                        ot[s][0:HD, :], bcs[:],
                        mybir.AluOpType.mult)

            noop = lambda: None
            ebs = []

            # ================= phase 1: qkv projections =================
            # chunk-0 attention (pure diagonal) is emitted mid-phase: its exp
            # stream runs on the otherwise idle ACT engine while PE grinds
            # through the projection matmuls.
            import contextlib
            _stack = contextlib.ExitStack()
            pxt = _stack.enter_context(tc.tile_pool(name="xt", bufs=1))
            if True:
                xt = [pxt.tile([P, T], BF16, tag=f"x{k}", name=f"x{k}") for k in range(DK)]
                wv_sb = [pxt.tile([P, G], BF16, tag=f"wv{k}", name=f"wv{k}") for k in range(DK)]
                wqk_sb = [pxt.tile([P, 1024], BF16, tag=f"wqk{k}", name=f"wqk{k}")
                          for k in range(DK)]
                # head loads: only x cols 0:512 gate the first v/qk blocks
                # (attention chunk 0 and all t<4 v-blocks live there); the
                # x remainder streams in under the projection compute.
                for k in range(DK):
                    nc.sync.dma_start(wv_sb[k][:], wv[k * P:(k + 1) * P, :])
                for k in range(DK):
                    nc.sync.dma_start(xt[k][:, 0:512],
                                      xT[k * P:(k + 1) * P, 0:512])
                for k in range(DK):
                    nc.sync.dma_start(wqk_sb[k][:], wqk[k * P:(k + 1) * P, :])
                for k in range(DK):
                    nc.sync.dma_start(xt[k][:, 512:T],
                                      xT[k * P:(k + 1) * P, 512:T])
                nc.sync.dma_start(mask_sb[:], bigmask[:])
                nc.sync.dma_start(ident[:], identin[:])
                nc.vector.tensor_copy(ident_bf[:], ident[:])
                nc.vector.tensor_copy(maskneg_bf[:], mask_sb[:, 384:512])
                for pb in range(4):
                    nc.sync.dma_start(wout_sb[pb][:], wout[pb * P:(pb + 1) * P, :])

                def v_block(t):
                    ps = psmm.tile([P, G], F32, tag="mm", name="vps")
                    for k in range(DK):
                        nc.tensor.matmul(
                            ps[:], xt[k][:, t * P:(t + 1) * P], wv_sb[k][:],
                            start=(k == 0), stop=(k == DK - 1))
                    nc.scalar.activation(
                        v1[t][:, :, 0:HD],
                        ps[:].rearrange("p (h c) -> p h c", h=8), Copy)
                    nc.vector.memset(v1[t][:, :, HD:HD + 1], 1.0)
                    if t < 12:
                        # fp8 copy for DoubleRow AV. t<4 runs on Pool
                        # (emitted before the gather burst); later blocks are
                        # emitted mid-attention where Pool is still chewing
                        # gather descriptors, so they go to DVE instead.
                        eng = nc.gpsimd if t < 4 else nc.vector
                        eng.tensor_copy(
                            v8[t // 2][:, :, t % 2, 0:HD], v1[t][:, :, 0:HD])

                def qk_block_c(jp, c):
                    ps = psmm.tile([P, 512], F32, tag="mm", name="qkps")
                    for k in range(DK):
                        nc.tensor.matmul(
                            ps[:], wqk_sb[k][:, jp * P:(jp + 1) * P],
                            xt[k][:, c * 512:(c + 1) * 512],
                            start=(k == 0), stop=(k == DK - 1))
                    nc.vector.tensor_copy(
                        qkT[jp][:, c * 512:(c + 1) * 512], ps[:])

                def gather_block(t):
                    idxt = pe.tile([P, KT], I32, tag="idx", name="idxt")
                    nc.sync.dma_start(idxt[:], seqs[t * P:(t + 1) * P, :])
                    gt = [pg.tile([P, G], F32, tag="g", name=f"g{kk}")
                          for kk in range(KT)]
                    for kk in range(KT):
                        nc.gpsimd.indirect_dma_start(
                            out=gt[kk][:], out_offset=None, in_=embs[:],
                            in_offset=bass.IndirectOffsetOnAxis(
                                ap=idxt[:, kk:kk + 1], axis=0))
                    ga = pg.tile([P, G], F32, tag="g", name="gs01")
                    nc.vector.tensor_add(ga[:], gt[0][:], gt[1][:])
                    gb = pg.tile([P, G], F32, tag="g", name="gs23")
                    nc.vector.tensor_add(gb[:], gt[2][:], gt[3][:])
                    eb = pe.tile([P, G], BF16, tag="eb", name="eb")
                    nc.vector.tensor_add(eb[:], ga[:], gb[:])
                    ebs.append(eb)

                for t in range(4):
                    v_block(t)
                for jp in (0, 4, 1, 5, 2, 6, 3, 7):
                    qk_block_c(jp, 0)
                for t in range(TB):
                    gather_block(t)

            # ============ phase 2: attention + bias + out-proj ============
            with (
                tc.tile_pool(name="w8p", bufs=5) as pw8,
                tc.tile_pool(name="yout", bufs=3) as py,
            ):
                # tail work (bias transpose-add + out-projection + store) is
                # queued as ~0.5us steps and interleaved between attention
                # units: keeps PE fed during the ACT-paced exp stream without
                # parking a large PE block in front of the next unit's score
                # matmuls (which would starve ACT).
                filler = []

                def queue_tail(t):
                    def step_tp():
                        for pb in range(4):
                            tp = psmm.tile([P, P], BF16, tag="mm", name="tp")
                            nc.tensor.transpose(
                                tp[:], ebs[t][:, pb * P:(pb + 1) * P],
                                ident_bf[:])
                            nc.vector.tensor_add(
                                aT[pb][:, t * P:(t + 1) * P],
                                aT[pb][:, t * P:(t + 1) * P], tp[:])
                    box = {}

                    def step_proj(ch, half):
                        def go():
                            if ch == 0 and half == 0:
                                box["ysb"] = py.tile([P, D], BF16, tag="y",
                                                     name="y")
                            if half == 0:
                                box[ch] = psmm.tile([P, 512], F32, tag="mm",
                                                    name="yps")
                            yp = box[ch]
                            for pb in (0, 1) if half == 0 else (2, 3):
                                nc.tensor.matmul(
                                    yp[:],
                                    aT[pb][:, t * P:(t + 1) * P],
                                    wout_sb[pb][:, ch * 512:(ch + 1) * 512],
                                    start=(pb == 0), stop=(pb == 3))
                            if half == 1:
                                ysb = box["ysb"]
                                nc.vector.tensor_copy(
                                    ysb[:, ch * 512:(ch + 1) * 512], yp[:])
                                if ch == 1:
                                    nc.sync.dma_start(y[t * P:(t + 1) * P, :],
                                                      ysb[:])
                        return go

                    filler.append(step_tp)
                    for ch in range(2):
                        filler.append(step_proj(ch, 0))
                        filler.append(step_proj(ch, 1))

                def emit_filler():
                    if filler:
                        filler.pop(0)()

                # projection work for later chunks, in dependency order:
                # attention chunk ic needs q-chunk ic, k-chunks <= ic and
                # v t-blocks <= 4*ic+3. Emitted as ~2us filler steps under
                # the ACT-paced exp stream; forced drains before each
                # emit_attn guarantee the deps are in the PE queue.
                need = {}
                for ic in range(1, TC):
                    for hp in range(4):
                        if hp == 0:
                            filler.append(lambda ic=ic: qk_block_c(0, ic))
                            filler.append(lambda ic=ic: qk_block_c(4, ic))
                            for t in range(4 * ic, 4 * ic + 4):
                                filler.append(lambda t=t: v_block(t))
                        else:
                            filler.append(
                                lambda hp=hp, ic=ic: qk_block_c(hp, ic))
                            filler.append(
                                lambda hp=hp, ic=ic: qk_block_c(4 + hp, ic))
                        need[(ic, hp)] = len(filler)
                emitted = [0]

                def emit_filler():
                    if filler:
                        filler.pop(0)()
                        emitted[0] += 1

                def drain_to(n):
                    while emitted[0] < n and filler:
                        emit_filler()

                for hp in range(4):
                    emit_attn(0, hp, pw8, emit_filler)
                for ic in range(1, TC):
                    for t in range(4 * (ic - 1), 4 * ic):
                        queue_tail(t)
                    for hp in range(4):
                        drain_to(need[(ic, hp)])
                        emit_attn(ic, hp, pw8, emit_filler)

                while filler:
                    emit_filler()
                qs = []
                for t in range(4 * (TC - 1), 4 * TC):
                    queue_tail(t)
                    qs.append(list(filler))
                    filler.clear()
                for step in range(max(len(q) for q in qs)):
                    for q in qs:
                        if step < len(q):
                            q[step]()
            _stack.close()

    nc.finalize()
    return nc


def _get_nc():
    global _NC_CACHE
    if _NC_CACHE is None:
        _NC_CACHE = _build()
    return _NC_CACHE


def _make_bigmask():
    # bigmask[jj, u] = NEG if u < jj + 384 else 0; mask for diagonal square of
    # block r is bigmask[:, 384:512] after shifting scores slice by r*128.
    jj = np.arange(P)[:, None]
    u = np.arange(896)[None, :]
    return np.where(u < jj + 384, np.float32(NEG), np.float32(0.0))


def _stage_inputs(x, sequences, Wqkv, Wout, emb, bias_scale, bout=None):
    x = np.asarray(x, dtype=np.float32)
    sequences = np.asarray(sequences)
    Wqkv = np.asarray(Wqkv, dtype=np.float32)
    Wout = np.asarray(Wout, dtype=np.float32)
    emb = np.asarray(emb, dtype=np.float32)
    bs = np.float32(np.asarray(bias_scale))

    bigmask = _make_bigmask()
    identm = np.eye(P, dtype=np.float32)

    in_maps = []
    for core in range(NCORES):
        b, g = divmod(core, 2)
        q_rows = slice(g * G, (g + 1) * G)
        k_rows = slice(D + g * G, D + (g + 1) * G)
        v_rows = slice(2 * D + g * G, 2 * D + (g + 1) * G)
        cols = slice(g * G, (g + 1) * G)
        in_maps.append(dict(
            xT=np.ascontiguousarray(x[b].T).astype(ml_dtypes.bfloat16),
            wqk=np.ascontiguousarray(
                np.concatenate([Wqkv[q_rows].T, Wqkv[k_rows].T],
                               axis=1)).astype(ml_dtypes.bfloat16),
            wv=np.ascontiguousarray(Wqkv[v_rows].T).astype(ml_dtypes.bfloat16),
            wout=np.ascontiguousarray(Wout[:, cols].T).astype(ml_dtypes.bfloat16),
            embs=np.ascontiguousarray(emb[:, cols] * bs),
            seqs=np.ascontiguousarray(sequences[b].astype(np.int32)),
            bigmask=bigmask,
            identin=identm,
        ))
    return in_maps


def _run(inputs, trace=False):
    nc = _get_nc()
    in_maps = _stage_inputs(**inputs)
    res = run_bass_kernel_spmd(nc, in_maps, core_ids=list(range(NCORES)),
                               trace=trace)
    bout = np.asarray(inputs["bout"], dtype=np.float32)
    out = np.empty((B, T, D), dtype=np.float32)
    for b in range(B):
        out[b] = (res.results[2 * b]["y"].astype(np.float32)
                  + res.results[2 * b + 1]["y"].astype(np.float32) + bout)
    return out, res


def kernel(**inputs):
    out, _ = _run(inputs, trace=False)
    return out


# revision 52
# speedup vs baseline: 1.0120x; 1.0056x over previous
"""Trainium2 Bass kernel for BaseAttention (B=4, T=2048, D=1024, H=16, K=4).

Sharding: 8 cores = 4 batches x 2 head-groups (8 heads each).
Per core: qkv projection (bf16 matmuls, fp32 accumulate), causal attention in
transposed-score layout (softmax denominators via a ones-column in the V
operand, causal mask folded into the PE as an identity x (-1e30 triangle)
accumulate), embedding gather bias (transposed on the PE), and a bf16
out-projection over this head-group's columns. Host sums the two partial
outputs per batch and adds bout.

Schedule: the attention phase is paced by the ACT-engine exp stream, so
 - chunk 0's attention (pure diagonal) is emitted inside the projection
   phase, hiding its exps under the PE-dense qkv matmuls;
 - off-diagonal attention runs in fp8e4 DoubleRow mode: score j-block pairs
   share one 2-bank PSUM tile, a single merged exp writes the fp8 weight
   pair, one DoubleRow matmul contracts 256 keys at 0.5 cyc/row (noise
   averages down over >=512 keys; diagonals stay bf16 for early tokens);
 - exp inputs are biased by -4 so unnormalized weights stay inside fp8e4
   range (max 240); the ones-column denominator absorbs the same factor;
 - bias-transpose + out-projection work is queued as ~0.5us steps and doled
   out between attention units as PE fill-in, never in front of the next
   unit's score matmuls.
"""
import sys

sys.path.insert(0, "/opt/trn_rl_repo")

import numpy as np
import ml_dtypes
import concourse.bass as bass
import concourse.mybir as mybir
import concourse.tile as tile
from concourse import bacc
from concourse.bass_utils import run_bass_kernel_spmd

P = 128
B, T, D, H = 4, 2048, 1024, 16
HD = D // H          # 64
KT = 4               # templates per token
VOCAB = 32000
G = 512              # columns per head-group (8 heads x 64)
NCORES = 8
TB = T // P          # 16 t-blocks of 128
TC = T // 512        # 4 t-chunks of 512
DK = D // P          # 8 contraction blocks
NEG = -1.0e30

F32 = mybir.dt.float32
BF16 = mybir.dt.bfloat16
F8 = mybir.dt.float8e4
I32 = mybir.dt.int32
Copy = mybir.ActivationFunctionType.Copy
Exp = mybir.ActivationFunctionType.Exp
DR = mybir.MatmulPerfMode.DoubleRow

_NC_CACHE = None


def _build():
    nc = bacc.Bacc("TRN2", target_bir_lowering=False, debug=False,
                   num_devices=NCORES)

    xT = nc.dram_tensor("xT", [D, T], BF16, kind="ExternalInput").ap()
    wqk = nc.dram_tensor("wqk", [D, 1024], BF16, kind="ExternalInput").ap()
    wv = nc.dram_tensor("wv", [D, G], BF16, kind="ExternalInput").ap()
    wout = nc.dram_tensor("wout", [G, D], BF16, kind="ExternalInput").ap()
    embs = nc.dram_tensor("embs", [VOCAB, G], F32, kind="ExternalInput").ap()
    seqs = nc.dram_tensor("seqs", [T, KT], I32, kind="ExternalInput").ap()
    bigmask = nc.dram_tensor("bigmask", [P, 896], F32, kind="ExternalInput").ap()
    identin = nc.dram_tensor("identin", [P, P], F32, kind="ExternalInput").ap()
    y = nc.dram_tensor("y", [T, D], BF16, kind="ExternalOutput").ap()

    with tile.TileContext(nc) as tc:
        with (
            tc.tile_pool(name="persist", bufs=1) as pp,
            tc.tile_pool(name="qk", bufs=1) as pqk,
            tc.tile_pool(name="v1", bufs=1) as pv1,
            tc.tile_pool(name="gath", bufs=7) as pg,
            tc.tile_pool(name="ebias", bufs=16) as pe,
            tc.tile_pool(name="work", bufs=1) as pw,
            tc.tile_pool(name="wtp", bufs=8) as pwt,
            tc.tile_pool(name="recp", bufs=2) as prec,
            tc.tile_pool(name="bcsp", bufs=4) as pbcs,
            tc.tile_pool(name="pssc", bufs=2, space="PSUM") as pssc,
            tc.tile_pool(name="psmm", bufs=2, space="PSUM") as psmm,
            tc.tile_pool(name="psot", bufs=2, space="PSUM") as psot,
        ):
            # ---- PE warmup spin during the DMA ramp: ~5us of back-to-back
            # matmuls on a zeroed tile pushes the HAM clock gate to 8/8 before
            # the real qkv matmuls start.
            wz = pp.tile([P, 512], BF16, tag="wz", name="wz")
            nc.vector.memset(wz[:], 0.0)
            wzp = psmm.tile([P, 512], F32, tag="mm", name="wzp")
            for w in range(24):
                nc.tensor.matmul(wzp[:], wz[:, 0:P], wz[:],
                                 start=(w == 0), stop=(w == 23))
            wzs = pp.tile([P, 512], F32, tag="wzs", name="wzs")
            nc.scalar.activation(wzs[:], wzp[:], Copy)

            # ---- persistent small tensors
            mask_sb = pp.tile([P, 896], F32, tag="mask", name="mask")
            ident = pp.tile([P, P], F32, tag="ident", name="ident")
            ones_sb = pp.tile([1, 64], BF16, tag="ones", name="ones")
            nc.vector.memset(ones_sb[:], 1.0)
            # exp bias -4: keeps unnormalized softmax weights inside fp8e4
            # range (max normal 240); the ones-column denominator picks up the
            # same e^-4 factor, so normalization cancels it exactly.
            cneg4 = pp.tile([P, 1], F32, tag="cneg4", name="cneg4")
            nc.vector.memset(cneg4[:], -4.0)
            ident_bf = pp.tile([P, P], BF16, tag="identbf", name="identbf")
            maskneg_bf = pp.tile([P, P], BF16, tag="masknegbf", name="masknegbf")
            wout_sb = [pp.tile([P, D], BF16, tag=f"wout{pb}", name=f"wout{pb}")
                       for pb in range(4)]

            # qkT: 8 pblocks (4 q + 4 k), [128 feat, T] bf16
            qkT = [pqk.tile([P, T], BF16, tag=f"qk{j}", name=f"qk{j}") for j in range(8)]
            # v1: 16 t-blocks [128, 8 heads, 65] bf16 (col 64 = ones)
            v1 = [pv1.tile([P, 8, HD + 1], BF16, tag=f"v{t}", name=f"v{t}") for t in range(TB)]
            # v8: fp8 copy for DoubleRow AV, paired t-blocks [key, head, jsub, 80]
            # (only t-blocks 0..11 are ever off-diagonal; inner dim padded
            # 65->80 because DoubleRow ldweights needs k-subtile step % 16 == 0)
            v8 = [pv1.tile([P, 8, 2, 80], F8, tag=f"v8{t}", name=f"v8{t}")
                  for t in range(6)]
            for t in range(6):
                nc.vector.memset(v8[t][:, :, :, HD:HD + 1], 1.0)
            # attention output, transposed [feat, T] bf16
            aT = [pw.tile([P, T], BF16, tag=f"a{pb}", name=f"a{pb}") for pb in range(4)]

            # ============ attention unit machinery ============
            def emit_attn(ic, hp, w8pool, emitfn):
                i0 = ic * 512
                qt, kt = qkT[hp], qkT[4 + hp]
                ot = [psot.tile([P, 512], F32, tag="ot", name=f"ot{s}")
                      for s in range(2)]
                npair = 2 * ic       # off-diagonal j-block pairs
                nunit = npair + 2    # + two diagonal double-units

                def scores(u):
                    wts = []
                    for s in range(2):
                        hbase = s * HD
                        sc = pssc.tile([P, 1024], F32, tag="sc", name="sc")
                        if u < npair:
                            for j in range(2):
                                jb = 2 * u + j
                                nc.tensor.matmul(
                                    sc[:, j * 512:(j + 1) * 512],
                                    kt[hbase:hbase + HD, jb * P:(jb + 1) * P],
                                    qt[hbase:hbase + HD, i0:i0 + 512],
                                    start=True, stop=True,
                                    tile_position=(hbase, 0))
                            # merged exp over both j-blocks, fp8 out
                            w8 = w8pool.tile([P, 2, 512], F8, tag="w8",
                                             name="w8")
                            nc.scalar.activation(
                                w8[:].rearrange("p a b -> p (a b)"),
                                sc[:], Exp, bias=cneg4[:], scale=0.125)
                            wts.append(w8)
                        else:
                            pairw = []
                            for j in range(2):
                                r = 2 * (u - npair) + j
                                lo = r * P
                                co = j * 512
                                nc.tensor.matmul(
                                    sc[:, co + lo:co + 512],
                                    kt[hbase:hbase + HD,
                                       (4 * ic + r) * P:(4 * ic + r + 1) * P],
                                    qt[hbase:hbase + HD, i0 + lo:i0 + 512],
                                    start=True, stop=True,
                                    tile_position=(hbase, 0))
                                nc.tensor.matmul(
                                    sc[:, co + lo:co + lo + P],
                                    ident_bf[:], maskneg_bf[:],
                                    start=False, stop=True,
                                    skip_group_check=True)
                                wt = pwt.tile([P, 512], BF16, tag="wt",
                                              name="wt")
                                nc.scalar.activation(
                                    wt[:, lo:512], sc[:, co + lo:co + 512],
                                    Exp, bias=cneg4[:], scale=0.125)
                                pairw.append((wt, lo, r))
                            wts.append(pairw)
                    return wts

                def accum(u, wts):
                    for s in range(2):
                        if u < npair:
                            nc.tensor.matmul(
                                ot[s][0:HD + 1, :],
                                v8[u][:, 2 * hp + s, :, 0:HD + 1],
                                wts[s][:],
                                start=(u == 0), stop=False,
                                perf_mode=DR)
                        else:
                            for (wt, lo, r) in wts[s]:
                                jb = 4 * ic + r
                                nc.tensor.matmul(
                                    ot[s][0:HD + 1, lo:512],
                                    v1[jb][:, 2 * hp + s, :],
                                    wt[:, lo:512],
                                    start=(u == 0 and r == 0),
                                    stop=(r == 3))

                prev = scores(0)
                for u in range(1, nunit):
                    cur = scores(u)
                    accum(u - 1, prev)
                    emitfn()
                    if len(filler) > 10:
                        emitfn()
                    prev = cur
                accum(nunit - 1, prev)
                # normalize: aT rows = o / denom (both reciprocals issued
                # first so the second's DVE latency hides under the first's
                # broadcast + multiply)
                recs = []
                for s in range(2):
                    # bf16: an fp32 moving operand costs 4.0 cyc/row on the
                    # PE (853ns vs 213ns for this broadcast matmul); bf16
                    # reciprocal quantization (~0.4%) scales a whole token's
                    # attention row uniformly, well inside the error budget.
                    rec = prec.tile([1, 512], BF16, tag="rec", name="rec")
                    with nc.allow_low_precision(reason="bf16 softmax recip"):
                        nc.vector.reciprocal(rec[:], ot[s][HD:HD + 1, :])
                    recs.append(rec)
                for s in range(2):
                    bc = psmm.tile([P, 512], F32, tag="mm", name="bc")
                    nc.tensor.matmul(bc[0:HD, :], ones_sb[:], recs[s][:],
                                     start=True, stop=True)
                    bcs = pbcs.tile([HD, 512], F32, tag="bcs", name="bcs")
                    nc.vector.tensor_copy(bcs[:], bc[0:HD, :])
                    nc.vector.tensor_tensor(
                        aT[hp][s * HD:(s + 1) * HD, i0:i0 + 512],
                        ot[s][0:HD, :], bcs[:],
                        mybir.AluOpType.mult)

            noop = lambda: None
            ebs = []

            # ================= phase 1: qkv projections =================
            # chunk-0 attention (pure diagonal) is emitted mid-phase: its exp
            # stream runs on the otherwise idle ACT engine while PE grinds
            # through the projection matmuls.
            import contextlib
            _stack = contextlib.ExitStack()
            pxt = _stack.enter_context(tc.tile_pool(name="xt", bufs=1))
            if True:
                xt = [pxt.tile([P, T], BF16, tag=f"x{k}", name=f"x{k}") for k in range(DK)]
                wv_sb = [pxt.tile([P, G], BF16, tag=f"wv{k}", name=f"wv{k}") for k in range(DK)]
                wqk_sb = [pxt.tile([P, 1024], BF16, tag=f"wqk{k}", name=f"wqk{k}")
                          for k in range(DK)]
                # head loads: only x cols 0:512 gate the first v/qk blocks
                # (attention chunk 0 and all t<4 v-blocks live there); the
                # x remainder streams in under the projection compute.
                for k in range(DK):
                    nc.sync.dma_start(wv_sb[k][:], wv[k * P:(k + 1) * P, :])
                for k in range(DK):
                    nc.sync.dma_start(xt[k][:, 0:512],
                                      xT[k * P:(k + 1) * P, 0:512])
                for k in range(DK):
                    nc.sync.dma_start(wqk_sb[k][:], wqk[k * P:(k + 1) * P, :])
                for k in range(DK):
                    nc.sync.dma_start(xt[k][:, 512:T],
                                      xT[k * P:(k + 1) * P, 512:T])
                nc.sync.dma_start(mask_sb[:], bigmask[:])
                nc.sync.dma_start(ident[:], identin[:])
                nc.vector.tensor_copy(ident_bf[:], ident[:])
                nc.vector.tensor_copy(maskneg_bf[:], mask_sb[:, 384:512])
                for pb in range(4):
                    nc.sync.dma_start(wout_sb[pb][:], wout[pb * P:(pb + 1) * P, :])

                def v_block(t):
                    ps = psmm.tile([P, G], F32, tag="mm", name="vps")
                    for k in range(DK):
                        nc.tensor.matmul(
                            ps[:], xt[k][:, t * P:(t + 1) * P], wv_sb[k][:],
                            start=(k == 0), stop=(k == DK - 1))
                    nc.scalar.activation(
                        v1[t][:, :, 0:HD],
                        ps[:].rearrange("p (h c) -> p h c", h=8), Copy)
                    nc.vector.memset(v1[t][:, :, HD:HD + 1], 1.0)
                    if t < 12:
                        # fp8 copy for DoubleRow AV. t<4 runs on Pool
                        # (emitted before the gather burst); later blocks are
                        # emitted mid-attention where Pool is still chewing
                        # gather descriptors, so they go to DVE instead.
                        eng = nc.gpsimd if t < 4 else nc.vector
                        eng.tensor_copy(
                            v8[t // 2][:, :, t % 2, 0:HD], v1[t][:, :, 0:HD])

                def qk_block_c(jp, c):
                    ps = psmm.tile([P, 512], F32, tag="mm", name="qkps")
                    for k in range(DK):
                        nc.tensor.matmul(
                            ps[:], wqk_sb[k][:, jp * P:(jp + 1) * P],
                            xt[k][:, c * 512:(c + 1) * 512],
                            start=(k == 0), stop=(k == DK - 1))
                    nc.vector.tensor_copy(
                        qkT[jp][:, c * 512:(c + 1) * 512], ps[:])

                def gather_block(t):
                    idxt = pe.tile([P, KT], I32, tag="idx", name="idxt")
                    nc.sync.dma_start(idxt[:], seqs[t * P:(t + 1) * P, :])
                    gt = [pg.tile([P, G], F32, tag="g", name=f"g{kk}")
                          for kk in range(KT)]
                    for kk in range(KT):
                        nc.gpsimd.indirect_dma_start(
                            out=gt[kk][:], out_offset=None, in_=embs[:],
                            in_offset=bass.IndirectOffsetOnAxis(
                                ap=idxt[:, kk:kk + 1], axis=0))
                    ga = pg.tile([P, G], F32, tag="g", name="gs01")
                    nc.vector.tensor_add(ga[:], gt[0][:], gt[1][:])
                    gb = pg.tile([P, G], F32, tag="g", name="gs23")
                    nc.vector.tensor_add(gb[:], gt[2][:], gt[3][:])
                    eb = pe.tile([P, G], BF16, tag="eb", name="eb")
                    nc.vector.tensor_add(eb[:], ga[:], gb[:])
                    ebs.append(eb)

                for t in range(4):
                    v_block(t)
                for jp in (0, 4, 1, 5, 2, 6, 3, 7):
                    qk_block_c(jp, 0)
                for t in range(TB):
                    gather_block(t)

            # ============ phase 2: attention + bias + out-proj ============
            with (
                tc.tile_pool(name="w8p", bufs=5) as pw8,
                tc.tile_pool(name="yout", bufs=3) as py,
            ):
                # tail work (bias transpose-add + out-projection + store) is
                # queued as ~0.5us steps and interleaved between attention
                # units: keeps PE fed during the ACT-paced exp stream without
                # parking a large PE block in front of the next unit's score
                # matmuls (which would starve ACT).
                filler = []

                def queue_tail(t):
                    def step_tp():
                        for pb in range(4):
                            tp = psmm.tile([P, P], BF16, tag="mm", name="tp")
                            nc.tensor.transpose(
                                tp[:], ebs[t][:, pb * P:(pb + 1) * P],
                                ident_bf[:])
                            nc.vector.tensor_add(
                                aT[pb][:, t * P:(t + 1) * P],
                                aT[pb][:, t * P:(t + 1) * P], tp[:])
                    box = {}

                    def step_proj(ch, half):
                        def go():
                            if ch == 0 and half == 0:
                                box["ysb"] = py.tile([P, D], BF16, tag="y",
                                                     name="y")
                            if half == 0:
                                box[ch] = psmm.tile([P, 512], F32, tag="mm",
                                                    name="yps")
                            yp = box[ch]
                            for pb in (0, 1) if half == 0 else (2, 3):
                                nc.tensor.matmul(
                                    yp[:],
                                    aT[pb][:, t * P:(t + 1) * P],
                                    wout_sb[pb][:, ch * 512:(ch + 1) * 512],
                                    start=(pb == 0), stop=(pb == 3))
                            if half == 1:
                                ysb = box["ysb"]
                                nc.vector.tensor_copy(
                                    ysb[:, ch * 512:(ch + 1) * 512], yp[:])
                                if ch == 1:
                                    nc.sync.dma_start(y[t * P:(t + 1) * P, :],
                                                      ysb[:])
                        return go

                    filler.append(step_tp)
                    for ch in range(2):
                        filler.append(step_proj(ch, 0))
                        filler.append(step_proj(ch, 1))

                def emit_filler():
                    if filler:
                        filler.pop(0)()

                # projection work for later chunks, in dependency order:
                # attention chunk ic needs q-chunk ic, k-chunks <= ic and
                # v t-blocks <= 4*ic+3. Emitted as ~2us filler steps under
                # the ACT-paced exp stream; forced drains before each
                # emit_attn guarantee the deps are in the PE queue.
                need = {}
                for ic in range(1, TC):
                    for hp in range(4):
                        if hp == 0:
                            filler.append(lambda ic=ic: qk_block_c(0, ic))
                            filler.append(lambda ic=ic: qk_block_c(4, ic))
                            for t in range(4 * ic, 4 * ic + 4):
                                filler.append(lambda t=t: v_block(t))
                        else:
                            filler.append(
                                lambda hp=hp, ic=ic: qk_block_c(hp, ic))
                            filler.append(
                                lambda hp=hp, ic=ic: qk_block_c(4 + hp, ic))
                        need[(ic, hp)] = len(filler)
                emitted = [0]

                def emit_filler():
                    if filler:
                        filler.pop(0)()
                        emitted[0] += 1

                def drain_to(n):
                    while emitted[0] < n and filler:
                        emit_filler()

                for hp in range(4):
                    emit_attn(0, hp, pw8, emit_filler)
                for ic in range(1, TC):
                    for t in range(4 * (ic - 1), 4 * ic):
                        queue_tail(t)
                    for hp in range(4):
                        drain_to(need[(ic, hp)])
                        emit_attn(ic, hp, pw8, emit_filler)

                while filler:
                    emit_filler()
                qs = []
                for t in range(4 * (TC - 1), 4 * TC):
                    queue_tail(t)
                    qs.append(list(filler))
                    filler.clear()
                for step in range(max(len(q) for q in qs)):
                    for q in qs:
                        if step < len(q):
                            q[step]()
            _stack.close()

    nc.finalize()
    return nc


def _get_nc():
    global _NC_CACHE
    if _NC_CACHE is None:
        _NC_CACHE = _build()
    return _NC_CACHE


def _make_bigmask():
    # bigmask[jj, u] = NEG if u < jj + 384 else 0; mask for diagonal square of
    # block r is bigmask[:, 384:512] after shifting scores slice by r*128.
    jj = np.arange(P)[:, None]
    u = np.arange(896)[None, :]
    return np.where(u < jj + 384, np.float32(NEG), np.float32(0.0))


def _stage_inputs(x, sequences, Wqkv, Wout, emb, bias_scale, bout=None):
    x = np.asarray(x, dtype=np.float32)
    sequences = np.asarray(sequences)
    Wqkv = np.asarray(Wqkv, dtype=np.float32)
    Wout = np.asarray(Wout, dtype=np.float32)
    emb = np.asarray(emb, dtype=np.float32)
    bs = np.float32(np.asarray(bias_scale))

    bigmask = _make_bigmask()
    identm = np.eye(P, dtype=np.float32)

    in_maps = []
    for core in range(NCORES):
        b, g = divmod(core, 2)
        q_rows = slice(g * G, (g + 1) * G)
        k_rows = slice(D + g * G, D + (g + 1) * G)
        v_rows = slice(2 * D + g * G, 2 * D + (g + 1) * G)
        cols = slice(g * G, (g + 1) * G)
        in_maps.append(dict(
            xT=np.ascontiguousarray(x[b].T).astype(ml_dtypes.bfloat16),
            wqk=np.ascontiguousarray(
                np.concatenate([Wqkv[q_rows].T, Wqkv[k_rows].T],
                               axis=1)).astype(ml_dtypes.bfloat16),
            wv=np.ascontiguousarray(Wqkv[v_rows].T).astype(ml_dtypes.bfloat16),
            wout=np.ascontiguousarray(Wout[:, cols].T).astype(ml_dtypes.bfloat16),
            embs=np.ascontiguousarray(emb[:, cols] * bs),
            seqs=np.ascontiguousarray(sequences[b].astype(np.int32)),
            bigmask=bigmask,
            identin=identm,
        ))
    return in_maps


def _run(inputs, trace=False):
    nc = _get_nc()
    in_maps = _stage_inputs(**inputs)
    res = run_bass_kernel_spmd(nc, in_maps, core_ids=list(range(NCORES)),
                               trace=trace)
    bout = np.asarray(inputs["bout"], dtype=np.float32)
    out = np.empty((B, T, D), dtype=np.float32)
    for b in range(B):
        out[b] = (res.results[2 * b]["y"].astype(np.float32)
                  + res.results[2 * b + 1]["y"].astype(np.float32) + bout)
    return out, res


def kernel(**inputs):
    out, _ = _run(inputs, trace=False)
    return out
